# revision 1
# baseline (speedup 1.0000x reference)
"""DRaGNet3D Trainium kernel: depth-map -> 3D points -> kNN graph -> 2x GAT -> NetVLAD.

Distribution: 8 cores = 2 batches x 4 row-quarters. Each core computes the
cdist+top-10 for its 2400 query rows (padded to 2432), GAT attention for its
rows, then the batch group of 4 cores exchanges via AllReduce-emulated
gathers (g1 features, VLAD partial sums, output-column partials).

The conv backbone in the reference is dead code (its output is discarded), so
it is not computed.
"""
import numpy as np
import types, sys

B, DH, DW = 2, 64, 150
N = DH * DW            # 9600
K = 10
NT_FULL = 75           # n-tiles per batch (9600/128)
QR = 2400              # valid query rows per core
QP = 2432              # padded query rows (19*128)
RT = 19                # row tiles per core
CT = 19                # s column tiles (18*512 + 384)
G = 8                  # top-k segments (mod-8 interleave)
SEG = N // G           # 1200
C1 = 128               # GAT1 width
C2 = 256               # GAT2 width
H1W = 130              # h1 gather row: 128 feat + es + ed
H1P = 192              # padded (x4B = 768, %256==0)
H2W = 258
H2P = 320
KC = 64                # vlad clusters

_cache = {}


def _fold(w, a_s, a_d, bias, res):
    w64 = w.astype(np.float64)
    ws = (w64 @ a_s.astype(np.float64)).astype(np.float32)[:, None]
    wd = (w64 @ a_d.astype(np.float64)).astype(np.float32)[:, None]
    wfull = np.concatenate([w.astype(np.float32), ws, wd, res.astype(np.float32)], 1)
    b64 = bias.astype(np.float64)
    brow = np.concatenate([
        bias.astype(np.float32),
        np.array([b64 @ a_s.astype(np.float64)], np.float32),
        np.array([b64 @ a_d.astype(np.float64)], np.float32),
        np.zeros(res.shape[1], np.float32)]).astype(np.float32)
    return wfull, brow


def _build_program():
    import concourse.bass as bass
    import concourse.mybir as mybir
    import concourse.tile as tile
    from concourse import bacc
    from concourse.bass import ds

    F32 = mybir.dt.float32
    U32 = mybir.dt.uint32
    I16 = mybir.dt.int16
    F32R = mybir.dt.float32r
    AF = mybir.ActivationFunctionType
    OP = mybir.AluOpType
    AX = mybir.AxisListType

    nc = bacc.Bacc("TRN2", target_bir_lowering=False, debug=False, num_devices=8)

    # ---- inputs (per core)
    din = {}
    def I(name, shape, dt=F32):
        din[name] = nc.dram_tensor(name, shape, dt, kind="ExternalInput")
        return din[name]

    r_full = I("r_full", [128, NT_FULL])
    cphi = I("cphi", [128, NT_FULL]); sphi = I("sphi", [128, NT_FULL])
    sth = I("sth", [128, NT_FULL]);  cth = I("cth", [128, NT_FULL])
    r_q = I("r_q", [128, RT])
    cphiq = I("cphiq", [128, RT]); sphiq = I("sphiq", [128, RT])
    sthq = I("sthq", [128, RT]);  cthq = I("cthq", [128, RT])
    w1full = I("w1full", [5, H1W])       # [x,y,z,1] -> [W1|ws1|wd1] (+bias row 3)
    w1q = I("w1q", [5, H2W])             # for L4q lhsT: 0.5*(W1|ws|wd|res1); row3 = -brow
    w2main = I("w2main", [128, H2W])
    w2bias = I("w2bias", [1, H2W])
    w2res = I("w2res", [128, C2])
    cw = I("cw", [C2, KC])
    cb = I("cb", [1, KC])
    cw2t = I("cw2t", [KC, C2])
    h1w_oq = I("h1w_oq", [16384, KC])    # per-core 64 output cols of reordered h1w
    h1b_oq = I("h1b_oq", [1, KC])
    gw = I("gw", [C2, C2])
    gb = I("gb", [1, C2])
    row_off = I("row_off", [1, 1], U32)  # q*2400
    o_off = I("o_off", [1, 1], U32)      # q*64
    out_y = nc.dram_tensor("out_y", [C2, 1], F32, kind="ExternalOutput")

    groups = [[0, 1, 2, 3], [4, 5, 6, 7]]

    with tile.TileContext(nc) as tc:
        from contextlib import ExitStack
        with tc.tile_pool(name="const", bufs=1) as cpool, \
             tc.tile_pool(name="dram", bufs=1, space="DRAM") as dram, \
             tc.tile_pool(name="keep", bufs=1) as keep, \
             tc.tile_pool(name="psumv", bufs=1, space="PSUM") as psumv:

            # ---------- registers for dynamic offsets
            roff = nc.alloc_register(mybir.EngineType.Pool, "roff")
            ooff = nc.alloc_register(mybir.EngineType.Pool, "ooff")
            offs = cpool.tile([1, 1], U32, tag="offs")
            nc.sync.dma_start(offs[:], row_off.ap())
            nc.gpsimd.reg_load(roff, offs[:])
            offs2 = cpool.tile([1, 1], U32, tag="offs2")
            nc.sync.dma_start(offs2[:], o_off.ap())
            nc.gpsimd.reg_load(ooff, offs2[:])
            ROFF = bass.RuntimeValue(roff)
            OOFF = bass.RuntimeValue(ooff)

            # ---------- load small consts
            def ctile(t, shape, dt=F32):
                x = cpool.tile(shape, dt, tag=t.name)
                nc.sync.dma_start(x[:], t.ap())
                return x
            w1full_s = cpool.tile([5, H1W], F32R, tag="w1full")
            nc.sync.dma_start(w1full_s[:], w1full.ap().bitcast(F32R))
            w1q_s = cpool.tile([5, H2W], F32R, tag="w1q")
            nc.sync.dma_start(w1q_s[:], w1q.ap().bitcast(F32R))
            w2main_s = cpool.tile([128, H2W], F32R, tag="w2main")
            nc.sync.dma_start(w2main_s[:], w2main.ap().bitcast(F32R))
            w2main_f = ctile(w2main, [128, H2W])
            w2bias_s = ctile(w2bias, [1, H2W])
            w2res_s = ctile(w2res, [128, C2])
            cw_s = cpool.tile([128, 2*KC], F32, tag="cw_s")
            nc.sync.dma_start(cw_s[:, 0:KC], cw.ap()[0:128, :])
            nc.sync.dma_start(cw_s[:, KC:2*KC], cw.ap()[128:256, :])
            cb_s = ctile(cb, [1, KC])
            cw2t_s = ctile(cw2t, [KC, C2])
            h1b_s = ctile(h1b_oq, [1, KC])
            gb_s = ctile(gb, [1, C2])
            ones_row = cpool.tile([1, 128], F32, tag="ones_row")
            nc.vector.memset(ones_row[:], 1.0)
            ones64 = cpool.tile([64, 64], F32, tag="ones64")
            nc.vector.memset(ones64[:], 1.0)
            ident = cpool.tile([128, 128], F32, tag="ident")
            from concourse.masks import make_identity
            make_identity(nc, ident[:])

            # ---------- build point tables [128, 75] then roundtrip to DRAM
            bstack = ExitStack()
            work = bstack.enter_context(tc.tile_pool(name="bwork", bufs=1))
            rfs = ctile(r_full, [128, NT_FULL])
            cps = ctile(cphi, [128, NT_FULL]); sps = ctile(sphi, [128, NT_FULL])
            sts = ctile(sth, [128, NT_FULL]); cts = ctile(cth, [128, NT_FULL])
            tt = work.tile([128, NT_FULL], F32, tag="bt")
            px = work.tile([128, NT_FULL], F32, tag="px")
            py = work.tile([128, NT_FULL], F32, tag="py")
            pz = work.tile([128, NT_FULL], F32, tag="pz")
            sq = work.tile([128, NT_FULL], F32, tag="sq")
            on = work.tile([128, NT_FULL], F32, tag="on")
            nc.vector.tensor_tensor(out=tt[:], in0=rfs[:], in1=cps[:], op=OP.mult)
            nc.vector.tensor_tensor(out=px[:], in0=tt[:], in1=sts[:], op=OP.mult)
            nc.vector.tensor_tensor(out=py[:], in0=rfs[:], in1=sps[:], op=OP.mult)
            nc.vector.tensor_tensor(out=pz[:], in0=tt[:], in1=cts[:], op=OP.mult)
            t2 = work.tile([128, NT_FULL], F32, tag="bt2")
            nc.vector.tensor_tensor(out=sq[:], in0=px[:], in1=px[:], op=OP.mult)
            nc.vector.tensor_tensor(out=t2[:], in0=py[:], in1=py[:], op=OP.mult)
            nc.vector.tensor_tensor(out=sq[:], in0=sq[:], in1=t2[:], op=OP.add)
            nc.vector.tensor_tensor(out=t2[:], in0=pz[:], in1=pz[:], op=OP.mult)
            nc.vector.tensor_tensor(out=sq[:], in0=sq[:], in1=t2[:], op=OP.add)
            nc.vector.memset(on[:], 1.0)
            pts_d = dram.tile([5, N], F32R)   # rows px,py,pz,sq,ones
            for i, srw in enumerate((px, py, pz, sq, on)):
                nc.sync.dma_start(pts_d[i:i+1, :].rearrange("o (p f) -> (o p) f", p=128), srw[:].bitcast(F32R))

            # query L4q [4, 2432]: rows 2px,2py,2pz,-1
            rqs = ctile(r_q, [128, RT])
            cpq = ctile(cphiq, [128, RT]); spq = ctile(sphiq, [128, RT])
            stq = ctile(sthq, [128, RT]); ctq = ctile(cthq, [128, RT])
            r2 = work.tile([128, RT], F32, tag="qr2")
            qx = work.tile([128, RT], F32, tag="qx")
            qy = work.tile([128, RT], F32, tag="qy")
            qz = work.tile([128, RT], F32, tag="qz")
            qm = work.tile([128, RT], F32, tag="qm")
            qt = work.tile([128, RT], F32, tag="qt")
            nc.vector.tensor_scalar_mul(r2[:], rqs[:], 2.0)
            nc.vector.tensor_tensor(out=qt[:], in0=r2[:], in1=cpq[:], op=OP.mult)
            nc.vector.tensor_tensor(out=qx[:], in0=qt[:], in1=stq[:], op=OP.mult)
            nc.vector.tensor_tensor(out=qy[:], in0=r2[:], in1=spq[:], op=OP.mult)
            nc.vector.tensor_tensor(out=qz[:], in0=qt[:], in1=ctq[:], op=OP.mult)
            nc.vector.memset(qm[:], -1.0)
            qzr = work.tile([128, RT], F32, tag="qzr")
            nc.vector.memset(qzr[:], 0.0)
            L4q_d = dram.tile([5, QP], F32R)
            for i, srw in enumerate((qx, qy, qz, qm, qzr)):
                nc.sync.dma_start(L4q_d[i:i+1, :].rearrange("o (p f) -> (o p) f", p=128), srw[:].bitcast(F32R))
            bstack.close()
            astack = ExitStack()
            big = astack.enter_context(tc.tile_pool(name="phA", bufs=1))
            spool = astack.enter_context(tc.tile_pool(name="sp", bufs=2))
            work = astack.enter_context(tc.tile_pool(name="wa", bufs=2))
            psA = astack.enter_context(tc.tile_pool(name="psA", bufs=1, space="PSUM"))
            psS = astack.enter_context(tc.tile_pool(name="psS", bufs=2, space="PSUM"))

            R4 = big.tile([5, N], F32R, tag="R4")
            L4 = big.tile([5, QP], F32R, tag="L4")
            nc.sync.dma_start(R4[:], pts_d[:])
            nc.sync.dma_start(L4[:], L4q_d[:])
            B4 = R4

            # ---------- h1 gather table [9600, 192] (cols 0:130 valid)
            h1_d = dram.tile([N, H1P], F32)
            for t in range(NT_FULL):
                ps = psA.tile([128, H1W], F32, tag="ps_h1")
                nc.tensor.matmul(ps[:], B4[:, t*128:(t+1)*128], w1full_s[:], start=True, stop=True)
                hb = work.tile([128, H1W], F32, tag="h1sb")
                nc.scalar.activation(hb[:], ps[:], AF.Copy)
                nc.sync.dma_start(h1_d[t*128:(t+1)*128, 0:H1W], hb[:])

            # own-row augmented h1 [m, 258] = [pts_q|1]@(W1|ws|wd|res)+brow, via L4q*w1q
            h1q = keep.tile([128, RT*H2W], F32, tag="h1q")
            for t in range(RT):
                ps = psA.tile([128, H2W], F32, tag="ps_h1q")
                nc.tensor.matmul(ps[:], L4[:, t*128:(t+1)*128], w1q_s[:], start=True, stop=True)
                nc.scalar.activation(h1q[:, t*H2W:(t+1)*H2W], ps[:], AF.Copy)

            sPre = ExitStack()
            psB = sPre.enter_context(tc.tile_pool(name="psB", bufs=2, space="PSUM"))
            g1t_all = keep.tile([128, RT*128], F32, tag="g1t")
            # ---------- Phase A: s matmuls + top-10 per row tile
            edges_all = keep.tile([128, RT*K], U32, tag="edges_all")
            for t in range(RT):
                s = spool.tile([128, N], F32, tag="s")
                for c in range(CT):
                    c0 = c * 512
                    cw_ = min(512, N - c0)
                    ps = psS.tile([128, 512], F32, tag="ps_s")
                    nc.tensor.matmul(ps[:, :cw_], L4[:, t*128:(t+1)*128], R4[:, c0:c0+cw_], start=True, stop=True)
                    nc.scalar.activation(s[:, c0:c0+cw_], ps[:, :cw_], AF.Copy)
                cand = work.tile([128, 64], F32, tag="cand")
                sv = s[:].rearrange("q (o k) -> q k o", k=G)
                for k in range(G):
                    nc.vector.max(out=cand[:, k*8:(k+1)*8], in_=sv[:, k, :])
                t16 = work.tile([128, 16], F32, tag="t16")
                zap = work.tile([128, 64], F32, tag="zap")
                nc.vector.max(out=t16[:, 0:8], in_=cand[:])
                nc.vector.match_replace(out=zap[:], in_to_replace=t16[:, 0:8], in_values=cand[:], imm_value=-3e38)
                nc.vector.max(out=t16[:, 8:16], in_=zap[:])
                ed = work.tile([128, 16], U32, tag="ed")
                nc.vector.max_index(out=ed[:, 0:8], in_max=t16[:, 0:8], in_values=s[:])
                nc.vector.max_index(out=ed[:, 8:16], in_max=t16[:, 8:16], in_values=s[:])
                nc.vector.tensor_copy(edges_all[:, t*K:(t+1)*K], ed[:, 0:K])
                ga_full = work.tile([128, K*H1P], F32, tag="ga1", bufs=1)
                ga = ga_full[:].rearrange("p (k c) -> p k c", k=K)
                for kk in range(K):
                    nc.gpsimd.indirect_dma_start(
                        out=ga[:, kk, :], out_offset=None, in_=h1_d[:],
                        in_offset=bass.IndirectOffsetOnAxis(ap=edges_all[:, t*K+kk:t*K+kk+1], axis=0))
                hq = h1q[:, t*H2W:(t+1)*H2W]
                logits = work.tile([128, K], F32, tag="lg1")
                nc.scalar.activation(logits[:], ga[:, :, C1], AF.Lrelu, bias=hq[:, C1+1:C1+2], alpha=0.2)
                mx = work.tile([128, 4], F32, tag="mx1")
                nc.vector.tensor_reduce(mx[:, 0:1], logits[:], axis=AX.X, op=OP.max)
                nc.vector.tensor_scalar_mul(mx[:, 1:2], mx[:, 0:1], -1.0)
                wv = work.tile([128, K], F32, tag="wv1")
                nc.scalar.activation(wv[:], logits[:], AF.Exp, bias=mx[:, 1:2])
                nc.vector.tensor_reduce(mx[:, 2:3], wv[:], axis=AX.X, op=OP.add)
                nc.vector.reciprocal(mx[:, 3:4], mx[:, 2:3])
                acc = work.tile([128, C1], F32, tag="acc1")
                acc2 = work.tile([128, C1], F32, tag="acc1b")
                nc.vector.tensor_scalar(out=acc[:], in0=ga[:, 0, 0:C1], scalar1=wv[:, 0:1], scalar2=None, op0=OP.mult)
                cur, nxt = acc, acc2
                for k in range(1, K):
                    nc.vector.scalar_tensor_tensor(out=nxt[:], in0=ga[:, k, 0:C1], scalar=wv[:, k:k+1],
                                                   in1=cur[:], op0=OP.mult, op1=OP.add)
                    cur, nxt = nxt, cur
                nc.vector.tensor_scalar_mul(nxt[:], cur[:], mx[:, 3:4])
                nc.vector.tensor_tensor(out=nxt[:], in0=nxt[:], in1=hq[:, C1+2:H2W], op=OP.add)
                g1 = work.tile([128, C1], F32, tag="g1")
                nc.scalar.activation(g1[:], nxt[:], AF.Relu)
                pst = psB.tile([128, 128], F32, tag="ps_tr")
                nc.tensor.transpose(out=pst[:], in_=g1[:], identity=ident[:])
                nc.scalar.activation(g1t_all[:, t*128:(t+1)*128], pst[:], AF.Copy)


            sPre.close()
            astack.close()
            _bc = ExitStack()
            work = _bc.enter_context(tc.tile_pool(name="wb", bufs=2))

            # ---------- AllReduce-emulated AllGather of g1T [128, 9600]
            g1t_in = dram.tile([128, N], F32)
            g1t_out = dram.tile([128, N], F32)
            zf = work.tile([128, 1216], F32, tag="zf")
            nc.vector.memset(zf[:], 0.0)
            for i in range(8):
                nc.sync.dma_start(g1t_in[:, i*1200:(i+1)*1200], zf[:, 0:1200])
            for t in range(RT):
                w_ = 128 if t < RT - 1 else 96
                nc.gpsimd.dma_start(g1t_in[:, ds(ROFF + t*128, w_)], g1t_all[:, t*128:t*128+w_])
            nc.gpsimd.collective_compute("AllReduce", OP.add, replica_groups=groups,
                                         ins=[g1t_in.opt()], outs=[g1t_out.opt()])

            # ---------- Phase C: h2 table + GAT2
            sC1 = ExitStack()
            psC1 = sC1.enter_context(tc.tile_pool(name="psC1", bufs=2, space="PSUM"))
            h2_d = dram.tile([N, H2P], F32)
            for t in range(NT_FULL):
                g1f = work.tile([128, 128], F32R, tag="g1f")
                nc.sync.dma_start(g1f[:], g1t_out[:, t*128:(t+1)*128].bitcast(F32R))
                ps = psC1.tile([128, H2W], F32, tag="ps_h2")
                nc.tensor.matmul(ps[:], g1f[:], w2main_s[:], start=True, stop=False)
                nc.tensor.matmul(ps[:], ones_row[:], w2bias_s[:], start=False, stop=True)
                hb = work.tile([128, H2W], F32, tag="h2sb")
                nc.scalar.activation(hb[:], ps[:], AF.Copy)
                nc.sync.dma_start(h2_d[t*128:(t+1)*128, 0:H2W], hb[:])

            sC1.close()
            sC2 = ExitStack()
            psC = sC2.enter_context(tc.tile_pool(name="psC2", bufs=1, space="PSUM"))
            g2 = keep.tile([128, RT*(C2+1)], F32, tag="g2")   # per tile: 256 + ones col
            act_all = keep.tile([128, RT*KC], F32, tag="act")
            for t in range(RT):
                # own ed2 + res2 from resident g1T
                g1to = g1t_all[:, t*128:(t+1)*128]
                ps = psC.tile([128, H2W], F32, tag="ps_h2q")
                nc.tensor.matmul(ps[:], g1to, w2main_f[:], start=True, stop=False)
                nc.tensor.matmul(ps[:], ones_row[:], w2bias_s[:], start=False, stop=True)
                ed2 = work.tile([128, 2], F32, tag="ed2")
                nc.scalar.activation(ed2[:], ps[:, C2:C2+2], AF.Copy)
                psr = psC.tile([128, C2], F32, tag="ps_r2")
                nc.tensor.matmul(psr[:], g1to, w2res_s[:], start=True, stop=True)
                gb_full = work.tile([128, K*H2P], F32, tag="ga")
                gb_ = gb_full[:].rearrange("p (k c) -> p k c", k=K)
                for kk in range(K):
                    nc.gpsimd.indirect_dma_start(
                        out=gb_[:, kk, :], out_offset=None, in_=h2_d[:],
                        in_offset=bass.IndirectOffsetOnAxis(ap=edges_all[:, t*K+kk:t*K+kk+1], axis=0))
                logits = work.tile([128, K], F32, tag="lg2")
                nc.scalar.activation(logits[:], gb_[:, :, C2], AF.Lrelu, bias=ed2[:, 1:2], alpha=0.2)
                mx = work.tile([128, 4], F32, tag="mx2")
                nc.vector.tensor_reduce(mx[:, 0:1], logits[:], axis=AX.X, op=OP.max)
                nc.vector.tensor_scalar_mul(mx[:, 1:2], mx[:, 0:1], -1.0)
                wv = work.tile([128, K], F32, tag="wv2")
                nc.scalar.activation(wv[:], logits[:], AF.Exp, bias=mx[:, 1:2])
                nc.vector.tensor_reduce(mx[:, 2:3], wv[:], axis=AX.X, op=OP.add)
                nc.vector.reciprocal(mx[:, 3:4], mx[:, 2:3])
                acc = work.tile([128, C2], F32, tag="acc2")
                acc2 = work.tile([128, C2], F32, tag="acc2b")
                nc.vector.tensor_scalar(out=acc[:], in0=gb_[:, 0, 0:C2], scalar1=wv[:, 0:1], scalar2=None, op0=OP.mult)
                cur, nxt = acc, acc2
                for k in range(1, K):
                    nc.vector.scalar_tensor_tensor(out=nxt[:], in0=gb_[:, k, 0:C2], scalar=wv[:, k:k+1],
                                                   in1=cur[:], op0=OP.mult, op1=OP.add)
                    cur, nxt = nxt, cur
                g2sl = g2[:, t*(C2+1):(t+1)*(C2+1)]
                nc.vector.tensor_scalar_mul(nxt[:], cur[:], mx[:, 3:4])
                psr_sb = work.tile([128, C2], F32, tag="psr_sb")
                nc.scalar.activation(psr_sb[:], psr[:], AF.Copy)
                nc.vector.tensor_tensor(out=g2sl[:, 0:C2], in0=nxt[:], in1=psr_sb[:], op=OP.add)
                nc.vector.memset(g2sl[:, C2:C2+1], 1.0)

                # vlad soft-assignment for this tile
                pst1 = psC.tile([128, 128], F32, tag="ps_g2t")
                pst2 = psC.tile([128, 128], F32, tag="ps_g2t2")
                nc.tensor.transpose(out=pst1[:], in_=g2sl[:, 0:128], identity=ident[:])
                nc.tensor.transpose(out=pst2[:], in_=g2sl[:, 128:256], identity=ident[:])
                g2t1 = work.tile([128, 128], F32, tag="g2t1")
                g2t2 = work.tile([128, 128], F32, tag="g2t2")
                nc.scalar.activation(g2t1[:], pst1[:], AF.Copy)
                nc.scalar.activation(g2t2[:], pst2[:], AF.Copy)
                psl = psC.tile([128, KC], F32, tag="ps_lgt")
                nc.tensor.matmul(psl[:], g2t1[:], cw_s[:, 0:KC], start=True, stop=False)
                nc.tensor.matmul(psl[:], g2t2[:], cw_s[:, KC:2*KC], start=False, stop=False)
                nc.tensor.matmul(psl[:], ones_row[:], cb_s[:], start=False, stop=True)
                lg = work.tile([128, KC], F32, tag="lgv")
                nc.scalar.activation(lg[:], psl[:], AF.Copy)
                vm = work.tile([128, 4], F32, tag="vmx")
                nc.vector.tensor_reduce(vm[:, 0:1], lg[:], axis=AX.X, op=OP.max)
                nc.vector.tensor_scalar_mul(vm[:, 1:2], vm[:, 0:1], -1.0)
                av = act_all[:, t*KC:(t+1)*KC]
                nc.scalar.activation(av[:], lg[:], AF.Exp, bias=vm[:, 1:2])
                nc.vector.tensor_reduce(vm[:, 2:3], av[:], axis=AX.X, op=OP.add)
                nc.vector.reciprocal(vm[:, 3:4], vm[:, 2:3])
                nc.vector.tensor_scalar_mul(av[:], av[:], vm[:, 3:4])
                if t == RT - 1:
                    nc.vector.memset(act_all[96:128, t*KC:(t+1)*KC], 0.0)

            sC2.close()
            sC3 = ExitStack()
            psC = sC3.enter_context(tc.tile_pool(name="psC3", bufs=1, space="PSUM"))
            # ---------- V-hat accumulation [64, 257]
            psv = psumv.tile([64, C2+1], F32, tag="ps_v")
            for t in range(RT):
                nc.tensor.matmul(psv[:], act_all[:, t*KC:(t+1)*KC], g2[:, t*(C2+1):(t+1)*(C2+1)],
                                 start=(t == 0), stop=(t == RT - 1))
            vhat = work.tile([64, C2+1], F32, tag="vhat")
            nc.scalar.activation(vhat[:], psv[:], AF.Copy)
            v_in = dram.tile([64, C2+1], F32)
            v_out = dram.tile([64, C2+1], F32)
            nc.gpsimd.dma_start(v_in[:], vhat[:])
            nc.gpsimd.collective_compute("AllReduce", OP.add, replica_groups=groups,
                                         ins=[v_in.opt()], outs=[v_out.opt()])
            V = work.tile([64, C2+1], F32, tag="V")
            nc.sync.dma_start(V[:], v_out[:])

            # ---------- VLAD tail (replicated per core)
            nega = work.tile([64, 1], F32, tag="nega")
            nc.vector.tensor_scalar_mul(nega[:], V[:, C2:C2+1], -1.0)
            vlad = work.tile([64, C2], F32, tag="vlad")
            nc.vector.scalar_tensor_tensor(out=vlad[:], in0=cw2t_s[:], scalar=nega[:], in1=V[:, 0:C2],
                                           op0=OP.mult, op1=OP.add)
            junk = work.tile([64, C2], F32, tag="junk")
            n2 = work.tile([64, 4], F32, tag="n2")
            nc.vector.scalar_tensor_tensor(out=junk[:], in0=vlad[:], scalar=1.0, in1=vlad[:],
                                           op0=OP.mult, op1=OP.mult, accum_out=n2[:, 0:1])
            nc.scalar.activation(n2[:, 1:2], n2[:, 0:1], AF.Sqrt)
            nc.vector.tensor_scalar_max(n2[:, 1:2], n2[:, 1:2], 1e-12)
            nc.vector.reciprocal(n2[:, 2:3], n2[:, 1:2])
            nc.vector.tensor_scalar_mul(vlad[:], vlad[:], n2[:, 2:3])
            nc.vector.scalar_tensor_tensor(out=junk[:], in0=vlad[:], scalar=1.0, in1=vlad[:],
                                           op0=OP.mult, op1=OP.mult, accum_out=n2[:, 3:4])
            pstot = psC.tile([64, 1], F32, tag="ps_tot")
            nc.tensor.matmul(pstot[:], ones64[:], n2[:, 3:4], start=True, stop=True)
            tot = work.tile([64, 4], F32, tag="tot")
            nc.scalar.activation(tot[:, 0:1], pstot[:], AF.Sqrt)
            nc.vector.tensor_scalar_max(tot[:, 0:1], tot[:, 0:1], 1e-12)
            nc.vector.reciprocal(tot[:, 1:2], tot[:, 0:1])
            nc.vector.tensor_scalar_mul(vlad[:], vlad[:], tot[:, 1:2])

            # transpose vlad -> vladT [128, 64] x2 halves
            pv1 = psC.tile([128, 64], F32, tag="ps_vt1")
            pv2 = psC.tile([128, 64], F32, tag="ps_vt2")
            nc.tensor.transpose(out=pv1[:], in_=vlad[:, 0:128], identity=ident[0:64, 0:64])
            nc.tensor.transpose(out=pv2[:], in_=vlad[:, 128:256], identity=ident[0:64, 0:64])
            vt1 = work.tile([128, 64], F32, tag="vt1")
            vt2 = work.tile([128, 64], F32, tag="vt2")
            nc.scalar.activation(vt1[:], pv1[:], AF.Copy)
            nc.scalar.activation(vt2[:], pv2[:], AF.Copy)

            # y partial: 64 output cols, contract over all 16384 (128 chunks)
            psy = psumv.tile([64, 1], F32, tag="ps_y")
            for i in range(128):
                kk, half = i // 2, i % 2
                vt = vt1 if half == 0 else vt2
                wchunk = work.tile([128, KC], F32, tag="h1w_sb")
                nc.sync.dma_start(wchunk[:], h1w_oq[i*128:(i+1)*128, :])
                nc.tensor.matmul(psy[:], wchunk[:], vt[:, kk:kk+1], start=(i == 0), stop=False)
            one11 = cpool.tile([1, 1], F32, tag="one11")
            nc.vector.memset(one11[:], 1.0)
            nc.tensor.matmul(psy[:], h1b_s[:], one11[:], start=False, stop=True)
            ypart = work.tile([64, 1], F32, tag="ypart")
            nc.scalar.activation(ypart[:], psy[:], AF.Copy)

            # gather y across group: [256, 1] bounce
            y_in = dram.tile([C2, 1], F32)
            y_out_d = dram.tile([C2, 1], F32)
            zf2 = work.tile([C2 // 2, 2], F32, tag="zf2")
            nc.vector.memset(zf2[:], 0.0)
            nc.sync.dma_start(y_in[:].rearrange("(a b) o -> a (b o)", a=128), zf2[:])
            nc.gpsimd.dma_start(y_in[ds(OOFF, 64), :], ypart[:])
            nc.gpsimd.collective_compute("AllReduce", OP.add, replica_groups=groups,
                                         ins=[y_in.opt()], outs=[y_out_d.opt()])
            yA = work.tile([128, 1], F32, tag="yA")
            yB = work.tile([128, 1], F32, tag="yB")
            nc.sync.dma_start(yA[:], y_out_d[0:128, :])
            nc.sync.dma_start(yB[:], y_out_d[128:256, :])

            # gates: z = y@gw + gb ; out = y*sigmoid(z)
            gw_s = keep.tile([128, 2*C2], F32, tag="gw_s")
            nc.sync.dma_start(gw_s[:, 0:C2], gw.ap()[0:128, :])
            nc.sync.dma_start(gw_s[:, C2:2*C2], gw.ap()[128:256, :])
            outs = []
            for half in range(2):
                psz = psC.tile([128, 1], F32, tag="ps_z")
                nc.tensor.matmul(psz[:], gw_s[:, half*128:half*128+128], yA[:], start=True, stop=False)
                nc.tensor.matmul(psz[:], gw_s[:, C2+half*128:C2+half*128+128], yB[:], start=False, stop=False)
                nc.tensor.matmul(psz[:], gb_s[:, half*128:(half+1)*128], one11[:], start=False, stop=True)
                gt = work.tile([128, 1], F32, tag="gt")
                nc.scalar.activation(gt[:], psz[:], AF.Sigmoid)
                yy = yA if half == 0 else yB
                oo = work.tile([128, 1], F32, tag="oo")
                nc.vector.tensor_tensor(out=oo[:], in0=yy[:], in1=gt[:], op=OP.mult)
                nc.sync.dma_start(out_y.ap()[half*128:(half+1)*128, :], oo[:])
            sC3.close()
            _bc.close()

    nc.compile()
    return nc


def _host_prep(inputs):
    """Build per-core input maps from full inputs."""
    import jax, jax.numpy as jnp
    cpu = jax.devices('cpu')[0]
    with jax.default_device(cpu):
        theta = jnp.linspace(-jnp.pi, jnp.pi, DW)
        phi = jnp.linspace(-jnp.pi/2, jnp.pi/2, DH)
        cphi = np.asarray(jnp.cos(phi), np.float32)
        sphi = np.asarray(jnp.sin(phi), np.float32)
        sth = np.asarray(jnp.sin(theta), np.float32)
        cth = np.asarray(jnp.cos(theta), np.float32)
    cphi_n = np.repeat(cphi, DW).astype(np.float32)   # per node n = dh*150+dw
    sphi_n = np.repeat(sphi, DW).astype(np.float32)
    sth_n = np.tile(sth, DH).astype(np.float32)
    cth_n = np.tile(cth, DH).astype(np.float32)

    w1full_, b1row = _fold(inputs['gat1_w'], inputs['gat1_as'], inputs['gat1_ad'],
                           inputs['gat1_b'], inputs['gat1_res'])
    # table version: rows x,y,z + bias row; only cols 0:130
    z1 = np.zeros((1, H1W), np.float32)
    w1full = np.concatenate([w1full_[:, 0:H1W], z1, b1row[None, 0:H1W]], 0).astype(np.float32)  # [5,130]
    # query version for L5q = [2x,2y,2z,-1,0]: 0.5*W rows, -bias row, zero row
    w1q = np.concatenate([0.5*w1full_, -b1row[None, :], np.zeros((1, w1full_.shape[1]), np.float32)], 0).astype(np.float32)  # [5,258]
    w2full_, b2row = _fold(inputs['gat2_w'], inputs['gat2_as'], inputs['gat2_ad'],
                           inputs['gat2_b'], inputs['gat2_res'])
    w2main = w2full_[:, 0:H2W].astype(np.float32)
    w2bias = b2row[None, 0:H2W].astype(np.float32)
    w2res = w2full_[:, H2W:H2W+C2].astype(np.float32)

    h1w = inputs['vlad_h1w'].astype(np.float32)   # [16384, 256], rows d*64+k
    ks, dsx = np.divmod(np.arange(KC*C2), C2)
    h1w_r = h1w[dsx*KC + ks, :]                   # rows k*256+d

    in_maps = []
    for c in range(8):
        b, q = c // 4, c % 4
        r = inputs['depth_map'][b].reshape(-1).astype(np.float32)
        rq = np.concatenate([r[q*QR:(q+1)*QR], np.zeros(QP-QR, np.float32)])
        def packq(v):
            vq = np.concatenate([v[q*QR:(q+1)*QR], np.zeros(QP-QR, np.float32)])
            return vq.reshape(128, RT).astype(np.float32)
        m = {
            'r_full': r.reshape(128, NT_FULL), 'cphi': cphi_n.reshape(128, NT_FULL),
            'sphi': sphi_n.reshape(128, NT_FULL), 'sth': sth_n.reshape(128, NT_FULL),
            'cth': cth_n.reshape(128, NT_FULL),
            'r_q': rq.reshape(128, RT), 'cphiq': packq(cphi_n), 'sphiq': packq(sphi_n),
            'sthq': packq(sth_n), 'cthq': packq(cth_n),
            'w1full': w1full, 'w1q': w1q, 'w2main': w2main, 'w2bias': w2bias,
            'w2res': w2res,
            'cw': inputs['vlad_cw'].astype(np.float32),
            'cb': inputs['vlad_cb'][None, :].astype(np.float32),
            'cw2t': inputs['vlad_cw2'][0].T.astype(np.float32).copy(),
            'h1w_oq': h1w_r[:, q*KC:(q+1)*KC].copy(),
            'h1b_oq': inputs['vlad_h1b'][None, q*KC:(q+1)*KC].astype(np.float32),
            'gw': inputs['gate_w'].astype(np.float32),
            'gb': inputs['gate_b'][None, :].astype(np.float32),
            'row_off': np.array([[q*QR]], np.uint32),
            'o_off': np.array([[q*KC]], np.uint32),
        }
        in_maps.append(m)
    return in_maps


def kernel(**inputs) -> np.ndarray:
    from concourse.bass_utils import run_bass_kernel_spmd
    if 'nc' not in _cache:
        _cache['nc'] = _build_program()
    nc = _cache['nc']
    in_maps = _host_prep(inputs)
    res = run_bass_kernel_spmd(nc, in_maps, core_ids=list(range(8)))
    out = np.stack([res.results[0]['out_y'][:, 0], res.results[4]['out_y'][:, 0]])
    return out.astype(np.float32)

